# revision 1
# baseline (speedup 1.0000x reference)
"""Trainium2 Bass kernel for CantorAttention (transposed-softmax scheme).

Strategy
--------
Sorting positions by Cantor value makes every query's 64-key route set live
inside a 384-wide, 128-aligned window of the sorted order (dense banded
attention after a host-side permutation).  All matmuls run in bf16 (fp32
psum accumulation); rel err ~4e-3 vs the 2e-2 gate.

Per core (batch b, 4-head block hb), everything feature-major ("T" = [feat, seq]):

  A:  qkT = wqk.T @ xT (+bias, ACT copy to bf16); V produced directly in
      [seq, head, dim] layout (stationary = xT chunk, moving = wv), with a
      per-head ones column appended -> V65 [128, 4, 65].
  C:  scoresT per (head, 128-query tile): mask matmul FIRST (start=True,
      whole [128,384] psum = one zero-region), then 3 K^T.Q chunk matmuls
      accumulate.  ACT exp writes bf16 straight into a chunk-major E store
      (col = 384*chunk + 128*tile, collision-free for this geometry) via a
      strided AP -- no PE transposes, no DVE prob copies.
  PV: lhsT = V65[c][:,h,:] (65 rows: 64 v-dims + ones), rhs = E slices ->
      po[65, 512] = [unnormalized attn outT; denominators].  Normalize after:
      ACT copy po->SBUF, DVE reciprocal of the den row, Pool (gpsimd)
      partition_broadcast, DVE multiply -> attn_outT bf16.
  D:  outp = wo.T @ attn_outT per 512-col block, interleaved with C so the
      tail is short.

Sharding: batch x head-block -> 8 cores (core c: b = c//4, heads 4*(c%4)..).
Host sums the 4 partial outT blocks per batch, transposes, un-permutes, +bias.
"""

import sys

sys.path.insert(0, "/opt/trn_rl_repo")

import numpy as np

B, S, DIM = 2, 2048, 1024
HEADS, DH = 16, 64
K_NEI = 64
N_CORES = 8
HPC = 4            # heads per core
QT = 128           # query tile (rows per tile)
NT = S // QT       # 16 query tiles
SUP = 4            # query tiles per supertile (512 queries)
NSUP = NT // SUP

_CACHE = {}


def _cantor_val(seq_len, depth=8):
    pos = np.arange(seq_len, dtype=np.float64)
    x = pos / max(1, seq_len - 1)
    x = np.clip(x, 1e-6, 1.0 - 1e-6)
    val = np.zeros_like(x)
    factor = 0.5
    for _ in range(depth):
        xs = x * 3.0
        digit = np.floor(xs)
        x = xs - digit
        val = val + (digit == 2.0).astype(np.float64) * factor
        factor *= 0.5
    return np.clip(val, 0.0, 1.0)


def _geometry(routes):
    """Window geometry from the runtime routes array."""
    val = _cantor_val(S)
    pi = np.argsort(val, kind="stable").astype(np.int64)
    rank = np.empty(S, np.int64)
    rank[pi] = np.arange(S)
    kr = rank[np.asarray(routes, np.int64)][pi]      # [S, K] key ranks, query-rank order
    lo = kr.min(1)
    hi = kr.max(1) + 1
    for win in (384, 512):
        a = np.zeros(NT, np.int64)
        ok = True
        for t in range(NT):
            l = int(lo[t * QT:(t + 1) * QT].min())
            h = int(hi[t * QT:(t + 1) * QT].max())
            a[t] = min(l // 128, (S - win) // 128)
            if h > a[t] * 128 + win:
                ok = False
                break
        if ok:
            return pi, rank, kr, a, win
    raise ValueError("routes structure incompatible with banded-window kernel")


def _chunk_meta(a, win):
    """chunk -> (first tile, last tile); E-store block layout col=NCH*c + t."""
    NCH = win // 128
    a = [int(v) for v in a]
    chunk_tiles = {}
    for t in range(NT):
        for j in range(NCH):
            c = a[t] + j
            lo_t, hi_t = chunk_tiles.get(c, (t, t))
            chunk_tiles[c] = (min(lo_t, t), max(hi_t, t))
    blocks = {(c, t) for t in range(NT) for j in range(NCH) for c in (a[t] + j,)}
    idx = {NCH * c + t for (c, t) in blocks}
    if len(idx) != len(blocks):
        raise ValueError("E-store linear layout collision for this geometry")
    e_blocks = max(idx) + 1
    return chunk_tiles, e_blocks


def _build_module(a, win, loop_n=1, phases="ACD", cheat_dma=False):
    from contextlib import nullcontext

    from concourse import bacc, tile, mybir
    from concourse.bass import AP

    f32 = mybir.dt.float32
    bf16 = mybir.dt.bfloat16
    AF = mybir.ActivationFunctionType
    ALU = mybir.AluOpType
    NCH = win // 128                      # chunks per window
    KAP = win                             # E-store chunk stride (cols)
    a = [int(v) for v in a]
    chunk_tiles, e_blocks = _chunk_meta(a, win)
    E_COLS = e_blocks * 128

    nc = bacc.Bacc("TRN2", target_bir_lowering=False, debug=False)
    xT = nc.dram_tensor("xT", [DIM, S], bf16, kind="ExternalInput").ap()
    wqk = nc.dram_tensor("wqk", [DIM, 512], bf16, kind="ExternalInput").ap()
    bqk = nc.dram_tensor("bqk", [512, 1], f32, kind="ExternalInput").ap()
    wv = nc.dram_tensor("wv", [DIM, 256], bf16, kind="ExternalInput").ap()
    wo = nc.dram_tensor("wo", [256, DIM], bf16, kind="ExternalInput").ap()
    maskT = nc.dram_tensor("maskT", [128, NT * win], bf16, kind="ExternalInput").ap()
    outp = nc.dram_tensor("outp", [DIM, S], bf16, kind="ExternalOutput").ap()

    with tile.TileContext(nc) as tc:
        with tc.tile_pool(name="persist", bufs=1) as pp:
            # Batched DMAs (the issuing sequencer is held for the whole
            # transfer + ~900ns sem overhead, so fewer/bigger is critical).
            # SP queue: wqk, wv, maskT, wo (+ xt n>=1 and even-nb stores,
            # emitted inside the loop).  ACT queue: xt0, bqk, odd-nb stores.
            wqk_sb = pp.tile([128, 8, 512], bf16)
            nc.sync.dma_start(out=wqk_sb,
                              in_=wqk.rearrange("(kk p) n -> p kk n", p=128))
            wv_sb = pp.tile([128, 8, 256], bf16)
            nc.sync.dma_start(out=wv_sb,
                              in_=wv.rearrange("(kk p) n -> p kk n", p=128))
            maskT_sb = pp.tile([128, NT * win], bf16)
            nc.sync.dma_start(out=maskT_sb, in_=maskT)
            wo_sb2 = pp.tile([128, 2, DIM], bf16)
            nc.sync.dma_start(out=wo_sb2,
                              in_=wo.rearrange("(p2 p) n -> p p2 n", p=128))
            bq_sb = pp.tile([128, 4], f32)
            nc.scalar.dma_start(out=bq_sb,
                                in_=bqk.rearrange("(m p) o -> p (m o)", p=128))

            qk_sb = [pp.tile([128, S], bf16, tag=f"qk{m}", name=f"qk{m}")
                     for m in range(4)]
            V65 = [pp.tile([128, HPC, 65], bf16, tag=f"V{cc}", name=f"V{cc}")
                   for cc in range(NT)]
            E_st = [pp.tile([128, E_COLS], bf16, tag=f"E{h}", name=f"E{h}")
                    for h in range(HPC)]
            attn_outT = [pp.tile([128, S], bf16, tag=f"aout{p}", name=f"aout{p}")
                         for p in range(2)]

            loop_cm = tc.For_i(0, loop_n, 1) if loop_n > 1 else nullcontext()
            with loop_cm:
                with tc.tile_pool(name="xt_pool", bufs=2) as pax, \
                     tc.tile_pool(name="st_pool", bufs=3) as pst, \
                     tc.tile_pool(name="rec_pool", bufs=2) as prc, \
                     tc.tile_pool(name="prb_pool", bufs=2) as prb_p, \
                     tc.tile_pool(name="psAV", bufs=2, space="PSUM") as psav, \
                     tc.tile_pool(name="psS", bufs=3, space="PSUM") as pss, \
                     tc.tile_pool(name="psPO", bufs=3, space="PSUM") as pspo:

                    xt_hold = [None]

                    def emit_A(n):
                        if "A" not in phases:
                            return
                        if cheat_dma and xt_hold[0] is not None:
                            xt = xt_hold[0]
                        else:
                            xt = pax.tile([128, 8, 512], bf16, tag="x",
                                          name=f"x_{n}")
                            q_eng = nc.scalar if n == 0 else nc.sync
                            q_eng.dma_start(
                                out=xt,
                                in_=xT.rearrange("(kk p) n -> p kk n", p=128)
                                      [:, :, n * 512:(n + 1) * 512])
                            xt_hold[0] = xt
                        for m in range(4):
                            ps = psav.tile([128, 512], f32, tag="av",
                                           name=f"psqk{m}_{n}")
                            for kk in range(8):
                                nc.tensor.matmul(
                                    ps, wqk_sb[:, kk, m * 128:(m + 1) * 128],
                                    xt[:, kk, :], start=(kk == 0), stop=(kk == 7))
                            nc.scalar.activation(
                                out=qk_sb[m][:, n * 512:(n + 1) * 512],
                                in_=ps, func=AF.Identity, bias=bq_sb[:, m:m + 1])
                        for ss in range(4):
                            cc = n * 4 + ss
                            ps = psav.tile([128, 512], f32, tag="av",
                                           name=f"psv{cc}")
                            psv = ps[:, 0:256]
                            for kk in range(8):
                                nc.tensor.matmul(
                                    psv, xt[:, kk, ss * 128:(ss + 1) * 128],
                                    wv_sb[:, kk, :], start=(kk == 0), stop=(kk == 7),
                                    skip_group_check=True)
                            nc.vector.tensor_copy(V65[cc][:, :, 0:64],
                                                  psv.rearrange("p (h d) -> p h d", h=4))
                            nc.gpsimd.memset(V65[cc][:, :, 64:65], 1.0)

                    def emit_s1(h, u):
                        qTh = qk_sb[h // 2]
                        kTh = qk_sb[2 + h // 2]
                        poff = (h % 2) * 64
                        for t in range(u * SUP, (u + 1) * SUP):
                            ps = pss.tile([128, 512], f32, tag="sc",
                                          name=f"sc{h}_{t}")
                            ps_s = ps[:, 0:win]
                            for j in range(NCH):
                                nc.tensor.matmul(
                                    ps_s[:, j * 128:(j + 1) * 128],
                                    kTh[poff:poff + 64,
                                        (a[t] + j) * 128:(a[t] + j + 1) * 128],
                                    qTh[poff:poff + 64, t * 128:(t + 1) * 128],
                                    start=(j == 0), stop=(j == NCH - 1),
                                    skip_group_check=True)
                            # exp -> E store, one strided op: block col = NCH*c + t
                            col0 = (NCH * a[t] + t) * 128
                            base = E_st[h][:, col0:col0 + 128]
                            out_ap = AP(base.tensor, base.offset,
                                        [list(base.ap[0]), [KAP, NCH], [1, 128]])
                            nc.scalar.activation(out=out_ap, in_=ps_s, func=AF.Exp)
                            # multiplicative {1,0} mask, in place on the E store
                            m01 = maskT_sb[:, t * win:(t + 1) * win].rearrange(
                                "p (j q) -> p j q", j=NCH)
                            eng = nc.vector if (h + t) % 2 == 0 else nc.gpsimd
                            eng.tensor_tensor(out_ap, out_ap, m01, ALU.mult)

                    def emit_s2(h, u):
                        # PV pieces: widest chunk start=True, straddlers split
                        tiles_u = range(u * SUP, (u + 1) * SUP)
                        chunks_u = sorted({a[t] + j for t in tiles_u
                                           for j in range(NCH)})
                        ranges = []
                        for c in chunks_u:
                            t0c, t1c = chunk_tiles[c]
                            tlo = max(t0c, u * SUP)
                            thi = min(t1c, (u + 1) * SUP - 1)
                            ranges.append((c, tlo * 128 - u * 512,
                                           (thi + 1) * 128 - u * 512))
                        first = max(ranges, key=lambda r: r[2] - r[1])
                        pieces = [first]
                        wlo, whi = first[1], first[2]
                        for c, o0, o1 in sorted(
                                (r for r in ranges if r is not first),
                                key=lambda r: r[1]):
                            for p0, p1 in ((o0, min(o1, wlo)),
                                           (max(o0, wlo), min(o1, whi)),
                                           (max(o0, whi), o1)):
                                if p1 > p0:
                                    pieces.append((c, p0, p1))
                            wlo, whi = min(wlo, o0), max(whi, o1)
                        po = pspo.tile([65, 512], f32, tag="po",
                                       name=f"po{h}_{u}")
                        for i_p, (c, o0, o1) in enumerate(pieces):
                            e0 = KAP * c + o0 + u * 512
                            nc.tensor.matmul(
                                po[:, o0:o1],
                                V65[c][:, h, :],
                                E_st[h][:, e0:e0 + (o1 - o0)],
                                start=(i_p == 0),
                                stop=(i_p == len(pieces) - 1),
                                skip_group_check=True)
                        rec = prc.tile([1, 512], f32, tag="rec",
                                       name=f"rec{h}_{u}")
                        nc.vector.reciprocal(rec, po[64:65, :])
                        prb = prb_p.tile([64, 512], f32, tag="prb",
                                         name=f"prb{h}_{u}")
                        nc.gpsimd.partition_broadcast(prb, rec)
                        poff = (h % 2) * 64
                        nc.vector.tensor_tensor(
                            attn_outT[h // 2][poff:poff + 64,
                                              u * 512:(u + 1) * 512],
                            po[0:64, :], prb, ALU.mult)

                    outp3 = outp.rearrange("(mm p) n -> p mm n", p=128)

                    def emit_D(nb):
                        if "D" not in phases:
                            return
                        for half in range(2):
                            st = pst.tile([128, 4, 512], bf16, tag="st",
                                          name=f"st{half}_{nb}")
                            for i in range(4):
                                mm = half * 4 + i
                                ps = psav.tile([128, 512], f32, tag="av",
                                               name=f"psd{mm}_{nb}")
                                for p2 in range(2):
                                    nc.tensor.matmul(
                                        ps, wo_sb2[:, p2, mm * 128:(mm + 1) * 128],
                                        attn_outT[p2][:, nb * 512:(nb + 1) * 512],
                                        start=(p2 == 0), stop=(p2 == 1))
                                if (mm + nb) % 2 == 0:
                                    nc.scalar.copy(st[:, i, :], ps)
                                else:
                                    nc.vector.tensor_copy(st[:, i, :], ps)
                            q_eng = nc.sync if nb % 2 == 0 else nc.scalar
                            q_eng.dma_start(
                                out=outp3[:, half * 4:(half + 1) * 4,
                                          nb * 512:(nb + 1) * 512],
                                in_=st)

                    emit_A(0)
                    emit_A(1)
                    if "C" in phases:
                        for h in range(HPC):
                            emit_s1(h, 0)
                        emit_A(2)
                        for h in range(HPC):
                            emit_s2(h, 0)
                            emit_s1(h, 1)
                        emit_A(3)
                        for h in range(HPC):
                            emit_s2(h, 1)
                            emit_s1(h, 2)
                        emit_D(0)
                        for h in range(HPC):
                            emit_s2(h, 2)
                            emit_s1(h, 3)
                        emit_D(1)
                        emit_D(2)
                        for h in range(HPC):
                            emit_s2(h, 3)
                        emit_D(3)
                    else:
                        emit_A(2)
                        emit_A(3)

    nc.compile()
    return nc


def _get_module(a, win):
    key = (tuple(int(v) for v in a), int(win))
    if key not in _CACHE:
        _CACHE[key] = _build_module(a, win)
    return _CACHE[key]


def _prepare_in_maps(x, routes, qkv_w, qkv_b, out_w, out_b):
    """Shared host-side prep: returns (in_maps, pi, a, win)."""
    import ml_dtypes

    bf = ml_dtypes.bfloat16
    x = np.ascontiguousarray(np.asarray(x, np.float32))
    qkv_w = np.asarray(qkv_w, np.float32)
    qkv_b = np.asarray(qkv_b, np.float32)
    out_w = np.asarray(out_w, np.float32)

    pi, rank, kr, a, win = _geometry(np.asarray(routes))
    NCH = win // 128
    SCALE = 1.0 / float(np.sqrt(DH))

    # maskT [128, NT*win]: maskT[p, t*win + j*128 + q] = multiplicative mask
    # for key (a[t]+j)*128+p, query t*128+q (1 selected / 0 not)
    mask3 = np.zeros((NT, QT, win), np.float32)
    rows = np.repeat(np.arange(QT), K_NEI)
    for t in range(NT):
        krt = (kr[t * QT:(t + 1) * QT] - a[t] * 128).ravel()
        mask3[t, rows, krt] = 1.0
    maskT_np = np.ascontiguousarray(
        mask3.reshape(NT, QT, NCH, 128).transpose(3, 0, 2, 1).reshape(128, NT * win)
    ).astype(bf)

    xT_b = [np.ascontiguousarray(x[b][pi].T).astype(bf) for b in range(B)]

    in_maps = []
    for c in range(N_CORES):
        b = c // (N_CORES // B)
        hb = c % (N_CORES // B)
        heads = range(hb * HPC, (hb + 1) * HPC)
        w_rows, b_rows = [], []
        for sect, scale in ((0, SCALE), (1, 1.0)):
            for h in heads:
                r0 = sect * DIM + h * DH
                w_rows.append(qkv_w[r0:r0 + DH] * scale)
                b_rows.append(qkv_b[r0:r0 + DH] * scale)
        wqk_c = np.ascontiguousarray(np.concatenate(w_rows, 0).T).astype(bf)
        bqk_c = np.concatenate(b_rows, 0).reshape(-1, 1).astype(np.float32)
        vr0 = 2 * DIM + hb * HPC * DH
        wv_c = np.ascontiguousarray(qkv_w[vr0:vr0 + 256].T).astype(bf)
        wo_c = np.ascontiguousarray(
            out_w[:, hb * HPC * DH:(hb + 1) * HPC * DH].T).astype(bf)
        in_maps.append({
            "xT": xT_b[b],
            "wqk": wqk_c,
            "bqk": bqk_c,
            "wv": wv_c,
            "wo": wo_c,
            "maskT": maskT_np,
        })
    return in_maps, pi, a, win


def kernel(x, routes, qkv_w, qkv_b, out_w, out_b):
    from concourse.bass_utils import run_bass_kernel_spmd

    out_b = np.asarray(out_b, np.float32)
    qkv_b = np.asarray(qkv_b, np.float32)
    out_w = np.asarray(out_w, np.float32)
    in_maps, pi, a, win = _prepare_in_maps(x, routes, qkv_w, qkv_b, out_w, out_b)

    nc = _get_module(a, win)
    res = run_bass_kernel_spmd(nc, in_maps, core_ids=list(range(N_CORES)))

    # v-bias contribution: probs sum to 1, so attn@(v+bv) = attn@v + bv and
    # outp picks up a constant wo.T @ bv per core -- add on host.
    bv_all = qkv_b[2 * DIM:3 * DIM]
    adj = np.zeros(DIM, np.float64)
    for hb in range(N_CORES // B):
        sl = slice(hb * HPC * DH, (hb + 1) * HPC * DH)
        wo_c = np.asarray(in_maps[hb]["wo"], np.float64)     # [256, DIM] (bf16-rounded)
        adj += bv_all[sl].astype(np.float64) @ wo_c

    out = np.empty((B, S, DIM), np.float32)
    for b in range(B):
        cores = [c for c in range(N_CORES) if c // (N_CORES // B) == b]
        outT = res.results[cores[0]]["outp"].astype(np.float32)
        for c in cores[1:]:
            outT = outT + res.results[c]["outp"].astype(np.float32)
        rows_sorted = outT.T                      # [S, DIM] in rank order
        tmp = np.empty_like(rows_sorted)
        tmp[pi] = rows_sorted
        out[b] = tmp + (out_b.astype(np.float64) + adj)[None, :].astype(np.float32)
    return out



# revision 7
# speedup vs baseline: 1.2186x; 1.2186x over previous
"""Trainium2 Bass kernel for CantorAttention (banded attention, fp8-residual
A-phase + transposed-PV softmax).

Scheme (per core: batch b, 4-head block hb; everything in Cantor-rank order):

  A:  QKV projection with fp8(e4m3) DoubleRow + residual compensation:
      x = x8 + xr8, w*64 = w8 + wr8 (residuals quantized straight into the
      denormal range -> absolute error ~2^-10 of parent, better than bf16).
      psum = x8.w8 + x8.wr8 + xr8.w8 at a single shared scale 64; the 1/64
      lands in the psum->SBUF copy (ACT scale port for qk, DVE/Pool
      tensor_scalar for V).  4 DoubleRow passes replace 8 bf16 passes.
  S:  per (head, 128-query tile): banded scores over the tile's 2-3 aligned
      128-key chunks (bf16, contraction 64), exp on ACT into a contiguous
      per-tile E block, {0,1} route mask multiplied in on DVE.
  PV: transposed: po4[q, 4*65] += E_chunk^T . V65 (65 cols/chunk, full
      128-row utilization); V's ones column makes col 64 the denominator.
  N:  DVE reciprocal [128,4] + per-head tensor_scalar -> attn [q, h, d] bf16;
      PE transpose (identity moving) -> attn_outT [hd, q]; copies on Pool.
  D:  out projection as in v1 (bf16, contraction 256), copies round-robin
      ACT/DVE/Pool, stores on the SP queue.

Sharding: batch x head-block -> 8 cores.  Host sums the 4 partial outT
blocks per batch, transposes, un-permutes, adds out/v biases.
"""

import sys

sys.path.insert(0, "/opt/trn_rl_repo")

import numpy as np

B, S, DIM = 2, 2048, 1024
HEADS, DH = 16, 64
K_NEI = 64
N_CORES = 8
HPC = 4            # heads per core
QT = 128           # query tile
NT = S // QT       # 16 query tiles
WSCALE = 64.0      # fp8 weight pre-scale

_CACHE = {}


def _cantor_val(seq_len, depth=8):
    pos = np.arange(seq_len, dtype=np.float64)
    x = pos / max(1, seq_len - 1)
    x = np.clip(x, 1e-6, 1.0 - 1e-6)
    val = np.zeros_like(x)
    factor = 0.5
    for _ in range(depth):
        xs = x * 3.0
        digit = np.floor(xs)
        x = xs - digit
        val = val + (digit == 2.0).astype(np.float64) * factor
        factor *= 0.5
    return np.clip(val, 0.0, 1.0)


def _geometry(routes):
    """Banded-window geometry: per query tile the 128-aligned key chunks
    [a[t], a[t]+nch[t]) covering all routed keys, plus the sequential E-store
    block offsets eoff[t] (in chunks)."""
    val = _cantor_val(S)
    pi = np.argsort(val, kind="stable").astype(np.int64)
    rank = np.empty(S, np.int64)
    rank[pi] = np.arange(S)
    kr = rank[np.asarray(routes, np.int64)][pi]      # [S, K] key ranks
    a = np.zeros(NT, np.int64)
    nch = np.zeros(NT, np.int64)
    for t in range(NT):
        lo = int(kr[t * QT:(t + 1) * QT].min())
        hi = int(kr[t * QT:(t + 1) * QT].max()) + 1
        a[t] = lo // 128
        nch[t] = -(-(hi - a[t] * 128) // 128)
        if nch[t] > 4:
            raise ValueError("routes structure incompatible with banded kernel")
    eoff = np.concatenate([[0], np.cumsum(nch)[:-1]])
    return pi, rank, kr, a, nch, eoff


def _build_module(a, nch, eoff, loop_n=1):
    from contextlib import nullcontext

    from concourse import bacc, tile, mybir

    f32 = mybir.dt.float32
    bf16 = mybir.dt.bfloat16
    f8 = mybir.dt.float8e4
    AF = mybir.ActivationFunctionType
    ALU = mybir.AluOpType
    DR = mybir.MatmulPerfMode.DoubleRow
    a = [int(v) for v in a]
    nch = [int(v) for v in nch]
    eoff = [int(v) for v in eoff]
    E_CH = eoff[-1] + nch[-1]              # total chunks (46)
    E_COLS = E_CH * 128

    nc = bacc.Bacc("TRN2", target_bir_lowering=False, debug=False)
    x8d = nc.dram_tensor("x8", [DIM, S], f8, kind="ExternalInput").ap()
    xr8d = nc.dram_tensor("xr8", [DIM, S], f8, kind="ExternalInput").ap()
    wqk8d = nc.dram_tensor("wqk8", [DIM, 512], f8, kind="ExternalInput").ap()
    wqkr8d = nc.dram_tensor("wqkr8", [DIM, 512], f8, kind="ExternalInput").ap()
    wv8d = nc.dram_tensor("wv8", [DIM, 256], f8, kind="ExternalInput").ap()
    wvr8d = nc.dram_tensor("wvr8", [DIM, 256], f8, kind="ExternalInput").ap()
    bqkd = nc.dram_tensor("bqk", [512, 1], f32, kind="ExternalInput").ap()
    wod = nc.dram_tensor("wo", [256, DIM], bf16, kind="ExternalInput").ap()
    maskTd = nc.dram_tensor("maskT", [128, E_COLS], bf16, kind="ExternalInput").ap()
    identd = nc.dram_tensor("ident", [128, 128], bf16, kind="ExternalInput").ap()
    outp = nc.dram_tensor("outp", [DIM, S], bf16, kind="ExternalOutput").ap()

    r8 = lambda t: t.rearrange("(kk p) n -> p kk n", p=128)

    with tile.TileContext(nc) as tc:
        with tc.tile_pool(name="persist", bufs=1) as pp:
            # Small/early DMAs on the ACT queue; bulk stream on SP.
            bq_sb = pp.tile([128, 4], f32)
            nc.scalar.dma_start(out=bq_sb,
                                in_=bqkd.rearrange("(m p) o -> p (m o)", p=128))
            ident_sb = pp.tile([128, 128], bf16)
            nc.scalar.dma_start(out=ident_sb, in_=identd)

            wqk8_sb = pp.tile([128, 8, 512], f8)
            nc.sync.dma_start(out=wqk8_sb, in_=r8(wqk8d))
            wqkr8_sb = pp.tile([128, 8, 512], f8)
            nc.sync.dma_start(out=wqkr8_sb, in_=r8(wqkr8d))
            wv8_sb = pp.tile([128, 8, 256], f8)
            nc.sync.dma_start(out=wv8_sb, in_=r8(wv8d))
            wvr8_sb = pp.tile([128, 8, 256], f8)
            nc.sync.dma_start(out=wvr8_sb, in_=r8(wvr8d))
            maskT_sb = pp.tile([128, E_COLS], bf16)
            nc.sync.dma_start(out=maskT_sb, in_=maskTd)
            wo_sb2 = pp.tile([128, 2, DIM], bf16)
            nc.sync.dma_start(out=wo_sb2,
                              in_=wod.rearrange("(p2 p) n -> p p2 n", p=128))

            qk_sb = [pp.tile([128, S], bf16, tag=f"qk{m}", name=f"qk{m}")
                     for m in range(4)]
            V65 = pp.tile([128, NT, HPC, 65], bf16, tag="V65", name="V65")
            E_st = pp.tile([128, HPC, E_COLS], bf16, tag="Est", name="Est")
            attn_outT = pp.tile([128, 2, S], bf16, tag="aout", name="aout")
            nc.gpsimd.memset(V65[:, :, :, 64:65], 1.0)

            loop_cm = tc.For_i(0, loop_n, 1) if loop_n > 1 else nullcontext()
            with loop_cm:
                with tc.tile_pool(name="xt_pool", bufs=2) as pax, \
                     tc.tile_pool(name="st_pool", bufs=3) as pst, \
                     tc.tile_pool(name="rec_pool", bufs=2) as prc, \
                     tc.tile_pool(name="at_pool", bufs=2) as pat, \
                     tc.tile_pool(name="psAV", bufs=2, space="PSUM") as psav, \
                     tc.tile_pool(name="psS", bufs=3, space="PSUM") as pss, \
                     tc.tile_pool(name="psPV", bufs=2, space="PSUM") as pspv, \
                     tc.tile_pool(name="psTR", bufs=1, space="PSUM") as pstr:

                    po4_hold = {}
                    tr_hold = {}

                    def emit_A(n):
                        x8t = pax.tile([128, 8, 512], f8, tag="x8", name=f"x8_{n}")
                        xr8t = pax.tile([128, 8, 512], f8, tag="xr8",
                                        name=f"xr8_{n}")
                        q_eng = nc.scalar if n == 0 else nc.sync
                        q_eng.dma_start(out=x8t,
                                        in_=r8(x8d)[:, :, n * 512:(n + 1) * 512])
                        q_eng.dma_start(out=xr8t,
                                        in_=r8(xr8d)[:, :, n * 512:(n + 1) * 512])
                        terms_qk = ((wqk8_sb, x8t), (wqkr8_sb, x8t), (wqk8_sb, xr8t))
                        for m in range(4):
                            ps = psav.tile([128, 512], f32, tag="av",
                                           name=f"psqk{m}_{n}")
                            i = 0
                            for wt, xt in terms_qk:
                                for p in range(4):
                                    nc.tensor.matmul(
                                        ps,
                                        wt[:, 2 * p:2 * p + 2, m * 128:(m + 1) * 128],
                                        xt[:, 2 * p:2 * p + 2, :],
                                        start=(i == 0), stop=(i == 11),
                                        perf_mode=DR)
                                    i += 1
                            nc.scalar.activation(
                                out=qk_sb[m][:, n * 512:(n + 1) * 512],
                                in_=ps, func=AF.Identity,
                                bias=bq_sb[:, m:m + 1], scale=1.0 / WSCALE)
                        terms_v = ((x8t, wv8_sb), (x8t, wvr8_sb), (xr8t, wv8_sb))
                        for ss in range(4):
                            cc = n * 4 + ss
                            ps = psav.tile([128, 512], f32, tag="av",
                                           name=f"psv{cc}")
                            psv = ps[:, 0:256]
                            i = 0
                            for xt, wt in terms_v:
                                for p in range(4):
                                    nc.tensor.matmul(
                                        psv,
                                        xt[:, 2 * p:2 * p + 2, ss * 128:(ss + 1) * 128],
                                        wt[:, 2 * p:2 * p + 2, :],
                                        start=(i == 0), stop=(i == 11),
                                        perf_mode=DR, skip_group_check=True)
                                    i += 1
                            if ss % 2 == 0:
                                nc.vector.tensor_scalar(
                                    V65[:, cc, :, 0:64],
                                    psv.rearrange("p (h d) -> p h d", h=4),
                                    1.0 / WSCALE, None, ALU.mult)
                            else:
                                nc.scalar.activation(
                                    out=V65[:, cc, :, 0:64],
                                    in_=psv.rearrange("p (h d) -> p h d", h=4),
                                    func=AF.Copy, scale=1.0 / WSCALE)

                    def emit_S(t):
                        """scores + exp + mask + PV for tile t."""
                        e0 = eoff[t] * 128
                        ncols = nch[t] * 128
                        po4 = pspv.tile([128, 512], f32, tag="po", name=f"po{t}")
                        po4_hold[t] = po4
                        for h in range(HPC):
                            poff = (h % 2) * 64
                            qT = qk_sb[h // 2]
                            kT = qk_sb[2 + h // 2]
                            ps = pss.tile([128, 512], f32, tag="sc",
                                          name=f"sc{h}_{t}")
                            for j in range(nch[t]):
                                nc.tensor.matmul(
                                    ps[:, j * 128:(j + 1) * 128],
                                    kT[poff:poff + 64,
                                       (a[t] + j) * 128:(a[t] + j + 1) * 128],
                                    qT[poff:poff + 64, t * 128:(t + 1) * 128],
                                    start=(j == 0), stop=(j == nch[t] - 1),
                                    skip_group_check=True)
                            nc.scalar.activation(
                                out=E_st[:, h, e0:e0 + ncols],
                                in_=ps[:, 0:ncols], func=AF.Exp)
                            eng = nc.gpsimd if h == 3 else nc.vector
                            eng.tensor_tensor(
                                E_st[:, h, e0:e0 + ncols],
                                E_st[:, h, e0:e0 + ncols],
                                maskT_sb[:, e0:e0 + ncols], ALU.mult)
                        nmm = HPC * nch[t]
                        i = 0
                        for h in range(HPC):
                            for j in range(nch[t]):
                                nc.tensor.matmul(
                                    po4[:, h * 65:h * 65 + 65],
                                    E_st[:, h, e0 + j * 128:e0 + (j + 1) * 128],
                                    V65[:, a[t] + j, h, :],
                                    start=(i == 0), stop=(i == nmm - 1),
                                    skip_group_check=True)
                                i += 1

                    def emit_N(t):
                        """normalize + transpose + copy-out for tile t."""
                        from concourse.bass import AP

                        po4 = po4_hold.pop(t)
                        den = AP(po4.tensor, po4.offset + 64,
                                 [list(po4.ap[0]), [65, 4]])
                        rec = prc.tile([128, 4], f32, tag="rec", name=f"rec{t}")
                        nc.vector.reciprocal(rec, den)
                        at = pat.tile([128, HPC, 64], bf16, tag="at",
                                      name=f"at{t}")
                        for h in range(HPC):
                            nc.vector.tensor_scalar(
                                at[:, h, :], po4[:, h * 65:h * 65 + 64],
                                rec[:, h:h + 1], None, ALU.mult)
                        tr = pstr.tile([128, 1024], bf16, tag="tr",
                                       name=f"tr{t}")
                        for i in range(2):
                            nc.tensor.matmul(
                                tr[:, i * 128:(i + 1) * 128],
                                at[:, 2 * i:2 * i + 2, :], ident_sb,
                                is_transpose=True,
                                start=(i == 0), stop=(i == 1),
                                skip_group_check=True)
                        nc.vector.tensor_copy(
                            attn_outT[:, :, t * 128:(t + 1) * 128],
                            tr[:, 0:256].rearrange("p (i q) -> p i q", i=2))

                    outp3 = outp.rearrange("(mm p) n -> p mm n", p=128)

                    def emit_D(nb):
                        for half in range(2):
                            st = pst.tile([128, 4, 512], bf16, tag="st",
                                          name=f"st{half}_{nb}")
                            for i in range(4):
                                mm = half * 4 + i
                                ps = psav.tile([128, 512], f32, tag="av",
                                               name=f"psd{mm}_{nb}")
                                for p2 in range(2):
                                    nc.tensor.matmul(
                                        ps, wo_sb2[:, p2, mm * 128:(mm + 1) * 128],
                                        attn_outT[:, p2, nb * 512:(nb + 1) * 512],
                                        start=(p2 == 0), stop=(p2 == 1))
                                if (mm + nb) % 2 == 0:
                                    nc.scalar.copy(st[:, i, :], ps)
                                else:
                                    nc.vector.tensor_copy(st[:, i, :], ps)
                            nc.sync.dma_start(
                                out=outp3[:, half * 4:(half + 1) * 4,
                                          nb * 512:(nb + 1) * 512],
                                in_=st)

                    # software-pipelined emission: N(t) lags S(t) by one tile
                    emit_A(0)
                    emit_A(1)
                    emit_S(0)
                    emit_S(1); emit_N(0)
                    emit_S(2); emit_N(1)
                    emit_A(2)
                    emit_S(3); emit_N(2)
                    emit_S(4); emit_N(3)
                    emit_S(5); emit_N(4)
                    emit_S(6); emit_N(5)
                    emit_A(3)
                    emit_S(7); emit_N(6)
                    emit_S(8); emit_N(7)
                    emit_S(9); emit_N(8)
                    emit_D(0)
                    emit_S(10); emit_N(9)
                    emit_S(11); emit_N(10)
                    emit_S(12); emit_N(11)
                    emit_D(1)
                    emit_S(13); emit_N(12)
                    emit_S(14); emit_N(13)
                    emit_S(15); emit_N(14)
                    emit_N(15)
                    emit_D(2)
                    emit_D(3)

    nc.compile()
    return nc


def _get_module(a, nch, eoff):
    key = (tuple(int(v) for v in a), tuple(int(v) for v in nch))
    if key not in _CACHE:
        _CACHE[key] = _build_module(a, nch, eoff)
    return _CACHE[key]


def _split8(arr):
    """x -> (x8, xr8) e4m3 pair with x ~= x8 + xr8 (residual hits denormals)."""
    import ml_dtypes

    f8 = ml_dtypes.float8_e4m3
    a = np.asarray(arr, np.float32)
    hi = a.astype(f8)
    lo = (a - hi.astype(np.float32)).astype(f8)
    return hi, lo


def _prepare_in_maps(x, routes, qkv_w, qkv_b, out_w, out_b):
    import ml_dtypes

    bf = ml_dtypes.bfloat16
    x = np.ascontiguousarray(np.asarray(x, np.float32))
    qkv_w = np.asarray(qkv_w, np.float32)
    qkv_b = np.asarray(qkv_b, np.float32)
    out_w = np.asarray(out_w, np.float32)

    pi, rank, kr, a, nch, eoff = _geometry(np.asarray(routes))
    E_CH = int(eoff[-1] + nch[-1])
    E_COLS = E_CH * 128
    SCALE = 1.0 / float(np.sqrt(DH))

    # maskT [128, E_COLS]: maskT[p, (eoff[t]+j)*128 + q] = 1 iff key
    # (a[t]+j)*128+p is routed for query t*128+q
    maskT_np = np.zeros((128, E_COLS), np.float32)
    for t in range(NT):
        krt = kr[t * QT:(t + 1) * QT] - a[t] * 128          # [128, K]
        qi = np.repeat(np.arange(QT), K_NEI)
        kk = krt.ravel()
        j, p = kk // 128, kk % 128
        maskT_np[p, (eoff[t] + j) * 128 + qi] = 1.0
    maskT_np = maskT_np.astype(bf)

    ident_np = np.eye(128, dtype=np.float32).astype(bf)

    x8_b, xr8_b = [], []
    for b in range(B):
        x8, xr8 = _split8(x[b][pi].T)
        x8_b.append(np.ascontiguousarray(x8))
        xr8_b.append(np.ascontiguousarray(xr8))

    in_maps = []
    for c in range(N_CORES):
        b = c // (N_CORES // B)
        hb = c % (N_CORES // B)
        heads = range(hb * HPC, (hb + 1) * HPC)
        w_cols, b_rows = [], []
        for sect, scale in ((0, SCALE), (1, 1.0)):
            for h in heads:
                r0 = sect * DIM + h * DH
                w_cols.append(qkv_w[r0:r0 + DH] * scale)
                b_rows.append(qkv_b[r0:r0 + DH] * scale)
        wqk = np.concatenate(w_cols, 0).T * WSCALE           # [1024, 512]
        wqk8, wqkr8 = _split8(wqk)
        bqk_c = np.concatenate(b_rows, 0).reshape(-1, 1).astype(np.float32)
        vr0 = 2 * DIM + hb * HPC * DH
        wv = qkv_w[vr0:vr0 + 256].T * WSCALE                 # [1024, 256]
        wv8, wvr8 = _split8(wv)
        wo_c = np.ascontiguousarray(
            out_w[:, hb * HPC * DH:(hb + 1) * HPC * DH].T).astype(bf)
        in_maps.append({
            "x8": x8_b[b], "xr8": xr8_b[b],
            "wqk8": np.ascontiguousarray(wqk8),
            "wqkr8": np.ascontiguousarray(wqkr8),
            "wv8": np.ascontiguousarray(wv8),
            "wvr8": np.ascontiguousarray(wvr8),
            "bqk": bqk_c,
            "wo": wo_c,
            "maskT": maskT_np,
            "ident": ident_np,
        })
    return in_maps, pi, (a, nch, eoff)


def kernel(x, routes, qkv_w, qkv_b, out_w, out_b):
    from concourse.bass_utils import run_bass_kernel_spmd

    out_b = np.asarray(out_b, np.float32)
    qkv_b = np.asarray(qkv_b, np.float32)
    out_w = np.asarray(out_w, np.float32)
    in_maps, pi, geom = _prepare_in_maps(x, routes, qkv_w, qkv_b, out_w, out_b)

    nc = _get_module(*geom)
    res = run_bass_kernel_spmd(nc, in_maps, core_ids=list(range(N_CORES)))

    # v-bias: probs sum to 1 -> attn@(v+bv) = attn@v + bv; outp picks up a
    # constant wo.T @ bv per core -- add on host (in the kernel's bf16 wo).
    bv_all = qkv_b[2 * DIM:3 * DIM]
    adj = np.zeros(DIM, np.float64)
    for hb in range(N_CORES // B):
        sl = slice(hb * HPC * DH, (hb + 1) * HPC * DH)
        wo_c = np.asarray(in_maps[hb]["wo"], np.float64)     # [256, DIM]
        adj += bv_all[sl].astype(np.float64) @ wo_c

    out = np.empty((B, S, DIM), np.float32)
    for b in range(B):
        cores = [c for c in range(N_CORES) if c // (N_CORES // B) == b]
        outT = res.results[cores[0]]["outp"].astype(np.float32)
        for c in cores[1:]:
            outT = outT + res.results[c]["outp"].astype(np.float32)
        rows_sorted = outT.T                      # [S, DIM] in rank order
        tmp = np.empty_like(rows_sorted)
        tmp[pi] = rows_sorted
        out[b] = tmp + (out_b.astype(np.float64) + adj)[None, :].astype(np.float32)
    return out
